# revision 25
# baseline (speedup 1.0000x reference)
"""Trainium2 Bass kernel for nn_MultiHeadCrossAttention.

Reference computation (B=2, S=2048, D=1024, H=16, HD=64):
  Qv,Kv,Vv = vis @ W_{q,k,v}_vis + b ; Qi,Ki,Vi = inf @ W_{q,k,v}_inf + b
  out_inf = softmax(Qv Ki^T / 8) Vi @ W_o_inf + b_o_inf
  out_vis = softmax(Qi Kv^T / 8) Vv @ W_o_vis + b_o_vis

Sharding: tensor-parallel over the 16 heads; core c owns heads 2c, 2c+1
(columns 128c:128c+128 of the QKV projections, rows of W_o). Each core
computes a full-shape partial of both outputs; the host sums the 8
partials (the "all-reduce after fc_out") and adds the output biases.

Device dataflow is fully transposed (token dim on the free axis) so no
on-device transposes of activations are needed except V:
  QT/KT/VT[j, t] = W.T @ X^T        (W stationary, X^T moving, 8 K-tiles)
  V = transpose(VT) via PE          (+ ones column -> V_aug [128k, 65])
  S^T[k, q]      = KT.T @ QT        (per head, K=64, row-group packed)
  E = exp(0.125 * S^T)              (ScalarE, PSUM -> SBUF bf16)
  PV[hd+1, q]    = V_aug.T @ E      (K=128; row 64 = softmax denominator)
  A^T[j, q]      = PV[:64] * bcast(1/PV[64])
  OUT^T[m, t]    = Wo.T @ A^T       (K=128, 8 m-tiles)
All matmuls bf16 with fp32 PSUM accumulation.
"""

import sys

for _p in ("/opt/trn_rl_repo", "/root/.axon_site/_ro/trn_rl_repo"):
    if _p not in sys.path:
        sys.path.append(_p)

import numpy as np
import ml_dtypes

import concourse.bass as bass
import concourse.tile as tile
from concourse import bacc, mybir
from concourse.masks import make_identity

F32 = mybir.dt.float32
BF16 = mybir.dt.bfloat16
EXP = mybir.ActivationFunctionType.Exp

B, S, D, H = 2, 2048, 1024, 16
HD = 64
JC = 128          # head dims per core (2 heads x 64)
N_CORES = 8
NT = 512          # token tile (moving dim) for projections / output
NQ = 512          # query tile for attention
KT = 128          # key tile (partition dim of S^T)
DKT = D // 128    # 8 contraction tiles for projections
SCALE = 1.0 / np.sqrt(HD)


def build_kernel():
    nc = bacc.Bacc()

    visT = nc.dram_tensor("visT", [B, D, S], BF16, kind="ExternalInput")
    infT = nc.dram_tensor("infT", [B, D, S], BF16, kind="ExternalInput")
    w_in = {}
    b_in = {}
    for st in ("v", "i"):
        for p in ("q", "k", "v"):
            w_in[p + st] = nc.dram_tensor(f"w_{p}{st}", [D, JC], BF16, kind="ExternalInput")
            b_in[p + st] = nc.dram_tensor(f"b_{p}{st}", [JC], F32, kind="ExternalInput")
    w_ov = nc.dram_tensor("w_ov", [JC, D], BF16, kind="ExternalInput")
    w_oi = nc.dram_tensor("w_oi", [JC, D], BF16, kind="ExternalInput")
    o_vis = nc.dram_tensor("o_vis", [B, D, S], F32, kind="ExternalOutput")
    o_inf = nc.dram_tensor("o_inf", [B, D, S], F32, kind="ExternalOutput")

    NTT = S // NT     # token tiles per batch
    NQT = S // NQ     # query tiles per batch
    NKT = S // KT     # key tiles per batch
    NMT = D // 128    # output m-tiles

    with tile.TileContext(nc) as tc:
        with (
            tc.tile_pool(name="const", bufs=1) as cpool,
            tc.tile_pool(name="wpool", bufs=1) as wpool,
            tc.tile_pool(name="proj", bufs=1) as projpool,   # QT/KT/VT/Vaug/AT per (st,b)
            tc.tile_pool(name="xin", bufs=3) as xpool,
            tc.tile_pool(name="esb", bufs=4) as epool,
            tc.tile_pool(name="small", bufs=4) as spool,
            tc.tile_pool(name="outst", bufs=4) as opool,
            tc.tile_pool(name="ps", bufs=1, space="PSUM") as ps,
        ):
            # ---- constants & weights (resident) ----
            ident = cpool.tile([128, 128], BF16)
            make_identity(nc, ident[:])

            # Weight/bias DMAs are emitted lazily at first use so the
            # first activation-tile DMA leads the queue and the PE can
            # start as early as possible.
            _w_tiles, _b_tiles, _wo_tiles = {}, {}, {}

            def w_sb_get(key):
                if key not in _w_tiles:
                    t = wpool.tile([128, DKT, JC], BF16, tag=f"w_{key}",
                                   name=f"w_{key}")
                    nc.sync.dma_start(
                        t[:], w_in[key].rearrange("(kt p) j -> p kt j", p=128))
                    _w_tiles[key] = t
                return _w_tiles[key]

            def bias_sb_get(key):
                if key not in _b_tiles:
                    t = cpool.tile([JC, 1], F32, tag=f"b_{key}", name=f"b_{key}")
                    nc.sync.dma_start(t[:], b_in[key][:].unsqueeze(1))
                    _b_tiles[key] = t
                return _b_tiles[key]

            def wo_sb_get(key):
                if key not in _wo_tiles:
                    wd = {"v": w_ov, "i": w_oi}[key]
                    t = wpool.tile([JC, NMT, 128], BF16, tag=f"wo_{key}",
                                   name=f"wo_{key}")
                    nc.sync.dma_start(
                        t[:], wd.rearrange("j (mt m) -> j mt m", m=128))
                    _wo_tiles[key] = t
                return _wo_tiles[key]

            xT = {"v": visT, "i": infT}
            o_dram = {"v": o_vis, "i": o_inf}

            # Deferred output projection: the previous query-tile's Wo
            # matmuls are interleaved one m-tile at a time into later
            # PE-dense loops, so the in-order PE never sits in a
            # low-duty stretch (which would re-throttle the HAM clock)
            # and never waits on the softmax-normalization chain.
            wo_tasks = []

            def pop_wo(n=1):
                for _ in range(n):
                    if not wo_tasks:
                        return
                    wo, mt, AT_, qsl_, od_, b_ = wo_tasks.pop(0)
                    po = ps.tile([128, NQ], F32, tag="proj", bufs=2,
                                 name="po")
                    nc.tensor.matmul(po[:], wo[:, mt, :], AT_[:, qsl_],
                                     start=True, stop=True)
                    ot = opool.tile([128, NQ], F32, tag="ot", name="ot")
                    nc.vector.tensor_copy(ot[:], po[:])
                    nc.sync.dma_start(
                        od_[b_, mt * 128:(mt + 1) * 128, qsl_], ot[:])

            def push_wo(wo, AT_, qsl_, od_, b_):
                for mt in range(NMT):
                    wo_tasks.append((wo, mt, AT_, qsl_, od_, b_))

            def flush_wo():
                pop_wo(len(wo_tasks))

            for b in range(B):
                # ---- projections for both streams ----
                qt_sb, kt_sb, vaug_sb = {}, {}, {}
                for st in ("v", "i"):
                    QT = projpool.tile([JC, S], BF16, tag=f"QT_{st}", bufs=2)
                    KTt = projpool.tile([JC, S], BF16, tag=f"KT_{st}", bufs=2)
                    VT = projpool.tile([JC, S], BF16, tag=f"VT_{st}", bufs=2)
                    qt_sb[st], kt_sb[st] = QT, KTt
                    dst = {"q": QT, "k": KTt, "v": VT}
                    for tt in range(NTT):
                        xt = xpool.tile([128, DKT, NT], BF16, tag="xt")
                        nc.sync.dma_start(
                            xt[:],
                            xT[st].rearrange("bb (kt p) t -> bb p kt t", p=128)[
                                b, :, :, tt * NT:(tt + 1) * NT],
                        )
                        for p in ("q", "k", "v"):
                            acc = ps.tile([128, NT], F32, tag="proj", bufs=2)
                            w = w_sb_get(p + st)
                            for kt in range(DKT):
                                nc.tensor.matmul(
                                    acc[:], w[:, kt, :], xt[:, kt, :],
                                    start=(kt == 0), stop=(kt == DKT - 1),
                                )
                            nc.vector.tensor_scalar_add(
                                dst[p][:, tt * NT:(tt + 1) * NT], acc[:],
                                bias_sb_get(p + st)[:],
                            )

                    # ---- V transpose + ones augmentation ----
                    Vaug = projpool.tile([128, NKT, 130], BF16,
                                         tag=f"Vaug_{st}", bufs=2)
                    vaug_sb[st] = Vaug
                    nc.vector.memset(Vaug[:, :, 64:65], 1.0)
                    nc.vector.memset(Vaug[:, :, 129:130], 1.0)
                    for k16 in range(NKT):
                        trp = ps.tile([128, 128], BF16, tag="proj", bufs=2)
                        nc.tensor.transpose(
                            trp[:], VT[:, k16 * 128:(k16 + 1) * 128], ident[:])
                        nc.vector.tensor_copy(Vaug[:, k16, 0:64], trp[:, 0:64])
                        nc.vector.tensor_copy(Vaug[:, k16, 65:129], trp[:, 64:128])

                # ---- attention: (query stream, kv stream, out) ----
                for qst, kvst, ost in (("v", "i", "i"), ("i", "v", "v")):
                    QT, KTt, Vaug = qt_sb[qst], kt_sb[kvst], vaug_sb[kvst]
                    AT = projpool.tile([JC, S], BF16, tag=f"AT_{ost}", bufs=2)
                    for qt in range(NQT):
                        qsl = slice(qt * NQ, (qt + 1) * NQ)
                        pv0 = ps.tile([65, NQ], F32, tag="pv0")
                        pv1 = ps.tile([65, NQ], F32, tag="pv1")
                        # software pipeline: S^T(k)+exp(k) run one key-tile
                        # ahead of PV(k) so the in-order PE never waits on
                        # the ScalarE exp.
                        sps = [None] * NKT
                        es = [None] * NKT

                        def stage_s(k16, _sps=sps, _es=es, _K=KTt, _Q=QT, _q=qsl):
                            ksl = slice(k16 * 128, (k16 + 1) * 128)
                            sp = ps.tile([128, 2, NQ], F32, tag="spair", bufs=2)
                            nc.tensor.matmul(sp[:, 0, :], _K[0:64, ksl], _Q[0:64, _q],
                                             start=True, stop=True)
                            nc.tensor.matmul(sp[:, 1, :], _K[64:128, ksl], _Q[64:128, _q],
                                             start=True, stop=True)
                            e01 = epool.tile([128, 2, NQ], BF16, tag="e01")
                            nc.scalar.activation(e01[:], sp[:], EXP, scale=SCALE)
                            _sps[k16], _es[k16] = sp, e01

                        def stage_pv(k16, _es=es, _V=Vaug, _pv0=pv0, _pv1=pv1):
                            e01 = _es[k16]
                            nc.tensor.matmul(_pv0[:], _V[:, k16, 0:65], e01[:, 0, :],
                                             start=(k16 == 0), stop=(k16 == NKT - 1))
                            nc.tensor.matmul(_pv1[:], _V[:, k16, 65:130], e01[:, 1, :],
                                             start=(k16 == 0), stop=(k16 == NKT - 1))

                        pop_wo(4)
                        stage_s(0)
                        pop_wo(1)
                        for k16 in range(1, NKT):
                            stage_s(k16)
                            stage_pv(k16 - 1)
                            if k16 % 2 == 1 and 3 <= k16 <= 7:
                                pop_wo(1)
                        stage_pv(NKT - 1)
                        # normalize: A^T = PV[:64] * bcast(1 / PV[64]).
                        # Denominators bounce PSUM->SBUF (raf can't read
                        # PSUM on HW), then one fast-reciprocal pass.
                        den = spool.tile([1, 2, NQ], F32, tag="den")
                        rec = spool.tile([1, 2, NQ], F32, tag="rec")
                        rb0 = spool.tile([64, NQ], F32, tag="rb0")
                        rb1 = spool.tile([64, NQ], F32, tag="rb1")
                        nc.vector.tensor_copy(den[0:1, 0, :], pv0[64:65, :])
                        nc.vector.reciprocal_approx_fast(rec[0:1, 0, :], den[0:1, 0, :])
                        nc.gpsimd.partition_broadcast(rb0[:, :], rec[0:1, 0, :])
                        nc.vector.tensor_copy(den[0:1, 1, :], pv1[64:65, :])
                        nc.vector.reciprocal_approx_fast(rec[0:1, 1, :], den[0:1, 1, :])
                        nc.gpsimd.partition_broadcast(rb1[:, :], rec[0:1, 1, :])
                        nc.vector.tensor_mul(AT[0:64, qsl], pv0[0:64, :], rb0[:, :])
                        nc.vector.tensor_mul(AT[64:128, qsl], pv1[0:64, :], rb1[:, :])
                        # queue this tile's output projection as PE filler
                        flush_wo()  # safety: clear any stragglers first
                        push_wo(wo_sb_get(ost), AT, qsl, o_dram[ost], b)

            flush_wo()

    nc.compile()
    return nc


_NC_CACHE = None


def _get_nc():
    global _NC_CACHE
    if _NC_CACHE is None:
        _NC_CACHE = build_kernel()
    return _NC_CACHE


def kernel(vis, inf, W_q_vis, b_q_vis, W_k_vis, b_k_vis, W_v_vis, b_v_vis,
           W_q_inf, b_q_inf, W_k_inf, b_k_inf, W_v_inf, b_v_inf,
           W_o_vis, b_o_vis, W_o_inf, b_o_inf):
    from concourse.bass_utils import run_bass_kernel_spmd

    nc = _get_nc()
    bf = ml_dtypes.bfloat16
    visT = np.ascontiguousarray(np.asarray(vis).transpose(0, 2, 1)).astype(bf)
    infT = np.ascontiguousarray(np.asarray(inf).transpose(0, 2, 1)).astype(bf)

    wq = {"v": np.asarray(W_q_vis), "i": np.asarray(W_q_inf)}
    wk = {"v": np.asarray(W_k_vis), "i": np.asarray(W_k_inf)}
    wv = {"v": np.asarray(W_v_vis), "i": np.asarray(W_v_inf)}
    bq = {"v": np.asarray(b_q_vis), "i": np.asarray(b_q_inf)}
    bk = {"v": np.asarray(b_k_vis), "i": np.asarray(b_k_inf)}
    bv = {"v": np.asarray(b_v_vis), "i": np.asarray(b_v_inf)}
    wo = {"v": np.asarray(W_o_vis), "i": np.asarray(W_o_inf)}

    in_maps = []
    for c in range(N_CORES):
        sl = slice(c * JC, (c + 1) * JC)
        m = {"visT": visT, "infT": infT}
        for st in ("v", "i"):
            m[f"w_q{st}"] = np.ascontiguousarray(wq[st][:, sl]).astype(bf)
            m[f"w_k{st}"] = np.ascontiguousarray(wk[st][:, sl]).astype(bf)
            m[f"w_v{st}"] = np.ascontiguousarray(wv[st][:, sl]).astype(bf)
            m[f"b_q{st}"] = np.ascontiguousarray(bq[st][sl]).astype(np.float32)
            m[f"b_k{st}"] = np.ascontiguousarray(bk[st][sl]).astype(np.float32)
            m[f"b_v{st}"] = np.ascontiguousarray(bv[st][sl]).astype(np.float32)
        m["w_ov"] = np.ascontiguousarray(wo["v"][sl, :]).astype(bf)
        m["w_oi"] = np.ascontiguousarray(wo["i"][sl, :]).astype(bf)
        in_maps.append(m)

    res = run_bass_kernel_spmd(nc, in_maps, list(range(N_CORES))).results

    ov = np.zeros((B, D, S), np.float32)
    oi = np.zeros((B, D, S), np.float32)
    for c in range(N_CORES):
        ov += res[c]["o_vis"]
        oi += res[c]["o_inf"]
    out_vis = ov.transpose(0, 2, 1) + np.asarray(b_o_vis)[None, None, :]
    out_inf = oi.transpose(0, 2, 1) + np.asarray(b_o_inf)[None, None, :]
    return (out_vis.astype(np.float32), out_inf.astype(np.float32))
